# revision 46
# baseline (speedup 1.0000x reference)
"""Trainium2 Bass kernel for NoTPAttention (dense transformer block:
fused QKV projection -> multi-head attention -> output projection).

Sharding (8 NeuronCores): core c handles batch b = c // 4 and the 4 heads
g = 4*(c % 4) .. 4*(c % 4)+3 (head-parallel tensor parallelism).  Each core
computes its heads' partial out-projection [S, H] in bf16; the host sums the
4 partials per batch in fp32 and adds the (folded) biases.

Numerics: matmuls run in bf16 with fp32 PSUM accumulation, except the Q and
K projections which run as fp8e4m3 DoubleRow (2 fp8 MACs/cell, ~1.7x;
weights pre-scaled x64 on the host, 1/64 folded into the PSUM drain).
q/k-quant errors only reach the output through softmax scores, measuring
rel err 0.0185 vs the 2e-2 gate (deterministic; k-only fallback via the
flags measures 0.0133).  v/out-proj fp8 fail the gate outright (their
errors pass through at full strength).  CRITICAL: the fp8 work must stay
interleaved with bf16 groups in <=~4.4us bursts — a dense fp8 block trips
the chip-wide P0 power downclock (2.4->2.0GHz, every matmul 375->450ns,
net slower).  Softmax is computed without max-subtraction (scores are
bounded, |s| < ~3.5) with the normalization deferred to the attention
*output*:
    attnT[d, q] = (sum_k v[k, d] * exp(sT[k, q])) / (sum_k exp(sT[k, q]))
The denominator is computed cheaply: the DVE pre-reduces the 16 key-tiles of
exp(sT) with a 4-level tree of bf16 adds ([128,16,512] -> [128,512]), and a
SINGLE ones-matmul per chunk does the remaining 128-partition sum, landing
the result already broadcast across partitions (16x less tensor-engine work
than ones-matmul-ing the full exp tensor).  The v-bias is dropped in-kernel:
after normalization it contributes exactly b_v to every row, so the host
folds w_out @ b_v into the output bias.

Pipeline: phase 2/3 run as 16 uniform iterations, each emitting (on PE):
  z-matmul(i-1) | ST pair 0,1 of chunk i+1 | PV(i) | 4 out-proj groups of
  the previous qc | ST pairs 2-7 of chunk i+1
~10.4us of tensor work per iteration vs ~9.2us of ACT exp, so the scalar
engine (1 elem/cycle/lane @1.2GHz, the hard exp floor) never becomes the
critical path.  Out-proj PSUM groups and the z matmul share the phase-1
"mm" PSUM tag so the total stays exactly 8 banks.  ST(0)/ST(1) are
interleaved into phase 1's last v-projections so exp warms up early.

Startup: inputs are stored partition-major in DRAM (host pre-swizzles) so
every DMA moves >=4KB packets (the DMA engines are packet-rate-bound at
~200ns/packet/queue); the first xc's q/k/v accumulation groups are 4-way
interleaved by ht so each arriving 0.5MB granule feeds 4 matmuls instead
of 1 and the HBM ramp is hidden (borrowing 2 idle "st" PSUM banks).

Layout notes: qT/kT/attnT live as [128 (head-dim), head, seq] so every
matmul contracts over a full 128-partition tile with no transposes anywhere.
The qkv weights share SBUF slots with the attention exp-buffers (tag "e"):
they are dead once the projections finish, exactly when the exp buffers
start rotating.
"""

import numpy as np
import ml_dtypes

B, S, H = 2, 2048, 2048
NH, HD = 16, 128
P = 128
HT = H // P            # 16 hidden-dim tiles
G = 4                  # heads per core
GH = G * HD            # 512: head-group width per core
SCALE = 1.0 / float(np.sqrt(HD))
N_CORES = 8
XC = 512               # phase-1 x streaming chunk (s elements)
QC = 512               # attention query chunk
KT = S // P            # 16 key tiles

_CACHE = {}

# q/k projections run as fp8e4m3 DoubleRow matmuls (2 fp8 MACs/cell, ~1.7x):
# their quantization errors reach the output only through softmax scores
# (measured rel err 0.0181 vs the 0.02 gate).  The v projection must stay
# bf16: v-path errors pass through to the output at full strength (v-fp8
# alone measures 0.022).  Flags allow falling back per-path.
Q_FP8 = True
K_FP8 = True


def _build():
    import concourse.mybir as mybir
    import concourse.tile as tile
    from concourse import bacc

    dt = mybir.dt
    Alu = mybir.AluOpType
    Act = mybir.ActivationFunctionType

    nc = bacc.Bacc("TRN2", target_bir_lowering=False, debug=False,
                   enable_asserts=False)

    NXC = S // XC      # 4
    NQC = S // QC      # 4

    # all inputs are stored partition-major in DRAM (host pre-swizzles; host
    # prep is free) so every partition's slice of a DMA is >=4KB contiguous:
    # the DMA engines are packet-rate-bound (~200ns/packet), so 1KB packets
    # from a [H, S]-layout x would make the startup ramp 4x slower.
    qk8 = Q_FP8 or K_FP8
    xt_r = nc.dram_tensor("xt", [P, NXC, HT, XC], dt.bfloat16,
                          kind="ExternalInput").ap()
    if qk8:
        xt8_r = nc.dram_tensor("xt8", [P, NXC, HT, XC], dt.float8e4,
                               kind="ExternalInput").ap()
    wqt_r = nc.dram_tensor("wqt", [P, HT, GH],
                           dt.float8e4 if Q_FP8 else dt.bfloat16,
                           kind="ExternalInput").ap()
    wkt_r = nc.dram_tensor("wkt", [P, HT, GH],
                           dt.float8e4 if K_FP8 else dt.bfloat16,
                           kind="ExternalInput").ap()
    wvt_r = nc.dram_tensor("wvt", [P, HT, GH], dt.bfloat16,
                           kind="ExternalInput").ap()
    bqs_d = nc.dram_tensor("bqs", [P, G], dt.float32, kind="ExternalInput").ap()
    bk_d = nc.dram_tensor("bk", [P, G], dt.float32, kind="ExternalInput").ap()
    wot_r = nc.dram_tensor("wot", [P, G, H], dt.bfloat16,
                           kind="ExternalInput").ap()
    out_d = nc.dram_tensor("partial", [S, H], dt.bfloat16,
                           kind="ExternalOutput").ap()

    with tile.TileContext(nc) as tc:
        with (
            tc.tile_pool(name="consts", bufs=1) as consts,
            tc.tile_pool(name="wpool", bufs=1) as wpool,
            tc.tile_pool(name="xpool", bufs=2) as xpool,
            tc.tile_pool(name="x8pool", bufs=2) as x8pool,
            tc.tile_pool(name="big", bufs=1) as big,
            tc.tile_pool(name="epool", bufs=3) as epool,
            tc.tile_pool(name="tree", bufs=1) as tpool,
            tc.tile_pool(name="espool", bufs=2) as espool,
            tc.tile_pool(name="small", bufs=2) as small,
            tc.tile_pool(name="psum", bufs=2, space="PSUM") as psum,
        ):
            # --- startup DMAs: finest-grained interleave of the wq and x
            # slices the very first accumulation group needs, so the first
            # matmul can start after minimal traffic.  With fp8 q/k the
            # startup-critical bytes halve (fp8 x + fp8 wq). ---
            wq_sb = epool.tile([P, HT, GH],
                               dt.float8e4 if Q_FP8 else dt.bfloat16,
                               tag="e", name="wq_sb")
            if qk8:
                x80_sb = x8pool.tile([P, HT, XC], dt.float8e4, tag="x8",
                                     name="x80_sb")
            xt0_sb = xpool.tile([P, HT, XC], dt.bfloat16, tag="xt",
                                name="xt0_sb")
            # 2-ht granules first so the very first accumulation matmuls can
            # start on ~0.5MB of traffic; coarser granules after.  The q
            # projection's own x form (fp8 or bf16) loads first; k's form
            # (if different) and the v projection's bf16 x follow.
            for hs in [slice(0, 2), slice(2, 4), slice(4, 8),
                       slice(8, 12), slice(12, 16)]:
                nc.sync.dma_start(wq_sb[:, hs, :], wqt_r[:, hs, :])
                if Q_FP8:
                    nc.sync.dma_start(x80_sb[:, hs, :], xt8_r[:, 0, hs, :])
                else:
                    nc.sync.dma_start(xt0_sb[:, hs, :], xt_r[:, 0, hs, :])
            wk_sb = epool.tile([P, HT, GH],
                               dt.float8e4 if K_FP8 else dt.bfloat16,
                               tag="e", name="wk_sb")
            wv_sb = epool.tile([P, HT, GH], dt.bfloat16, tag="e", name="wv_sb")
            nc.sync.dma_start(wk_sb[:, 0:4, :], wkt_r[:, 0:4, :])
            bqs_sb = consts.tile([P, G], dt.float32)
            nc.sync.dma_start(bqs_sb[:], bqs_d)
            bk_sb = consts.tile([P, G], dt.float32)
            nc.sync.dma_start(bk_sb[:], bk_d)
            ones_sb = consts.tile([P, P], dt.bfloat16)
            nc.vector.memset(ones_sb[:], 1.0)
            for b4 in range(1, 4):
                hs = slice(4 * b4, 4 * (b4 + 1))
                nc.sync.dma_start(wk_sb[:, hs, :], wkt_r[:, hs, :])
            for b4 in range(4):
                hs = slice(4 * b4, 4 * (b4 + 1))
                if K_FP8 and not Q_FP8:
                    nc.sync.dma_start(x80_sb[:, hs, :], xt8_r[:, 0, hs, :])
                if Q_FP8:   # v's bf16 x wasn't loaded by the q stream
                    nc.sync.dma_start(xt0_sb[:, hs, :], xt_r[:, 0, hs, :])
            for b4 in range(4):
                hs = slice(4 * b4, 4 * (b4 + 1))
                nc.sync.dma_start(wv_sb[:, hs, :], wvt_r[:, hs, :])

            qt_sb = big.tile([P, G, S], dt.bfloat16)   # q^T, scale+bias applied
            kt_sb = big.tile([P, G, S], dt.bfloat16)   # k^T, bias applied
            v_sb = big.tile([P, KT, GH], dt.bfloat16)  # v natural [s, o]
            at_sb = big.tile([P, G, S], dt.bfloat16)   # attn output^T

            chunks = [(h, qc) for qc in range(NQC) for h in range(G)]
            NCH = len(chunks)

            # ---------- phase 2 emit helpers (defined early: ST(0) is ----
            # ---------- interleaved into phase 1's last v-projections) ----
            e_tiles = {}
            es_tiles = {}
            pv_tiles = {}
            zi_tiles = {}

            def emit_st_pair(i, km):
                # ST^T = k^T.T @ q^T for key tiles km, km+1; exp on ACT in a
                # 2-bank batch (halves the 352-cycle per-ACTIVATE overhead).
                h, qc = chunks[i]
                if km == 0:
                    e_tiles[i] = epool.tile([P, KT, QC], dt.bfloat16, tag="e",
                                            name="e_sb")
                e_sb = e_tiles[i]
                ps = psum.tile([P, 2, QC], dt.float32, tag="st")
                for j in range(2):
                    nc.tensor.matmul(ps[:, j, :],
                                     kt_sb[:, h, (km + j) * P:(km + j + 1) * P],
                                     qt_sb[:, h, qc * QC:(qc + 1) * QC],
                                     start=True, stop=True)
                nc.scalar.activation(e_sb[:, km:km + 2, :], ps, Act.Exp)

            def emit_pv(i):
                h, qc = chunks[i]
                pv = psum.tile([P, QC], dt.float32, tag="pv")
                for km in range(KT):
                    nc.tensor.matmul(pv, v_sb[:, km, h * HD:(h + 1) * HD],
                                     e_tiles[i][:, km, :],
                                     start=(km == 0), stop=(km == KT - 1))
                pv_tiles[i] = pv

            def emit_tree(i):
                # KT-axis pre-reduction of exp(sT) on the DVE: 4 levels of
                # contiguous bf16 adds, [128,16,512] -> [128,512].
                e_sb = e_tiles[i]
                t1 = tpool.tile([P, 8, QC], dt.bfloat16, tag="t1")
                t2 = tpool.tile([P, 4, QC], dt.bfloat16, tag="t2")
                t3 = tpool.tile([P, 2, QC], dt.bfloat16, tag="t3")
                es = espool.tile([P, QC], dt.bfloat16, tag="es", name="es_sb")
                nc.vector.tensor_add(t1[:], e_sb[:, 0:8, :], e_sb[:, 8:16, :])
                nc.vector.tensor_add(t2[:], t1[:, 0:4, :], t1[:, 4:8, :])
                nc.vector.tensor_add(t3[:], t2[:, 0:2, :], t2[:, 2:4, :])
                nc.vector.tensor_add(es[:], t3[:, 0, :], t3[:, 1, :])
                es_tiles[i] = es

            def emit_tree_incremental(i, interleave=()):
                # last chunk: tree emitted in exp-delivery order so only ~4
                # small adds (not the whole 4us tree) trail the final exp;
                # `interleave` callbacks (the epilogue-feeding drain copies)
                # are sprinkled between the halves so neither blocks the
                # other in the DVE FIFO.
                e_sb = e_tiles[i]
                t1 = tpool.tile([P, 8, QC], dt.bfloat16, tag="t1")
                t2 = tpool.tile([P, 4, QC], dt.bfloat16, tag="t2")
                t3 = tpool.tile([P, 2, QC], dt.bfloat16, tag="t3")
                il = list(interleave)

                def pair(j):
                    nc.vector.tensor_add(t1[:, j, :],
                                         e_sb[:, 2 * j, :], e_sb[:, 2 * j + 1, :])

                for half in range(2):
                    o = 4 * half
                    pair(o); pair(o + 1)
                    nc.vector.tensor_add(t2[:, o // 2, :],
                                         t1[:, o, :], t1[:, o + 1, :])
                    if il:
                        il.pop(0)()
                    pair(o + 2); pair(o + 3)
                    nc.vector.tensor_add(t2[:, o // 2 + 1, :],
                                         t1[:, o + 2, :], t1[:, o + 3, :])
                    nc.vector.tensor_add(t3[:, half, :],
                                         t2[:, o // 2, :], t2[:, o // 2 + 1, :])
                for fn in il:
                    fn()
                # no final es add: the last znorm sums the two t3 halves
                # with two accumulating matmuls instead (shorter tail chain)
                es_tiles[i] = t3

            def emit_znorm(i):
                # single ones-matmul finishes the softmax denominator: sums
                # the 128 partitions of es and lands z broadcast in PSUM.
                h, qc = chunks[i]
                z = psum.tile([P, QC], dt.float32, tag="mm")
                es = es_tiles[i]
                if i == NCH - 1:
                    # last chunk: es was left as two tree halves; sum them
                    # here with two accumulating matmuls so the first fires
                    # before the final exp pair even lands.
                    nc.tensor.matmul(z, ones_sb[:], es[:, 0, :],
                                     start=True, stop=False)
                    nc.tensor.matmul(z, ones_sb[:], es[:, 1, :],
                                     start=False, stop=True)
                else:
                    nc.tensor.matmul(z, ones_sb[:], es[:],
                                     start=True, stop=True)
                zi = small.tile([P, QC], dt.float32, tag="zi")
                nc.vector.reciprocal_approx_fast(out=zi[:], in_=z)
                nc.vector.tensor_mul(out=at_sb[:, h, qc * QC:(qc + 1) * QC],
                                     in0=pv_tiles[i], in1=zi[:])

            ob_tiles = {}

            def emit_proj_group(qc, grp, last=False):
                # one out-proj PSUM group: accumulate the 4 heads for one
                # (seq-tile, out-col) block and drain it.  In the epilogue
                # (last=True) the pv banks are free: alternate tags so the
                # PSUM rotation isn't gated by the 0.7us drain copies.  The
                # 4 oc-blocks of one seq-tile drain into one ob tile and
                # leave as a single 512KB DMA with 4KB-contiguous rows (1KB
                # packets made the bare final drains take ~8us).
                sv, oc = grp // 4, grp % 4
                sm = qc * (QC // P) + sv
                tag = ("pv" if grp % 2 else "mm") if last else "mm"
                pp = psum.tile([P, 512], dt.float32, tag=tag)
                for g in range(G):
                    nc.tensor.matmul(pp,
                                     at_sb[:, g, sm * P:(sm + 1) * P],
                                     wo_sb[:, g, oc * 512:(oc + 1) * 512],
                                     start=(g == 0), stop=(g == G - 1))
                if oc == 0:
                    ob_tiles[sm] = small.tile([P, G, 512], dt.bfloat16,
                                              tag="ob", name="ob_sb")
                ob = ob_tiles[sm]
                # in the final (post-pipeline) groups ACT is idle: split the
                # drain copies across DVE and ACT so the tail isn't
                # serialized on one engine.
                if last and grp % 2 == 1:
                    nc.scalar.copy(ob[:, oc, :], pp)
                else:
                    nc.vector.tensor_copy(out=ob[:, oc, :], in_=pp)
                if last and sm == S // P - 1:
                    # the very last seq-tile's DMA is fully exposed: issue
                    # per-oc pieces as each drain copy lands so transfer
                    # pipelines with drain production.
                    nc.sync.dma_start(
                        out_d[sm * P:(sm + 1) * P, oc * 512:(oc + 1) * 512],
                        ob[:, oc, :])
                elif oc == G - 1:
                    # one 512KB DMA per seq-tile; its packets round-robin
                    # across all 16 queues on their own, so no manual split
                    nc.sync.dma_start(out_d[sm * P:(sm + 1) * P, :], ob[:])

            # ---------------- Phase 1: QKV projections ----------------
            DR = mybir.MatmulPerfMode.DoubleRow

            def qk_mms(ps, w_sb, h, x8_sb, xt_sb, fp8, ht_step=None):
                # emit one ht-accumulation step (or all, if ht_step is None)
                # of a q/k projection group: fp8 DoubleRow pairs two
                # 128-deep contractions per matmul (~1.7x).
                if fp8:
                    steps = range(HT // 2) if ht_step is None else [ht_step]
                    for t in steps:
                        nc.tensor.matmul(
                            ps, w_sb[:, 2 * t:2 * t + 2, h * HD:(h + 1) * HD],
                            x8_sb[:, 2 * t:2 * t + 2, :],
                            start=(t == 0), stop=(t == HT // 2 - 1),
                            perf_mode=DR)
                else:
                    steps = (range(HT) if ht_step is None
                             else [2 * ht_step, 2 * ht_step + 1])
                    for ht in steps:
                        nc.tensor.matmul(
                            ps, w_sb[:, ht, h * HD:(h + 1) * HD],
                            xt_sb[:, ht, :],
                            start=(ht == 0), stop=(ht == HT - 1))

            def drain_q(h, sl, ps):
                nc.vector.tensor_scalar(qt_sb[:, h, sl], ps,
                                        SCALE / 64.0 if Q_FP8 else SCALE,
                                        bqs_sb[:, h:h + 1], Alu.mult, Alu.add)

            def drain_k(h, sl, ps):
                if K_FP8:
                    nc.vector.tensor_scalar(kt_sb[:, h, sl], ps, 1.0 / 64.0,
                                            bk_sb[:, h:h + 1],
                                            Alu.mult, Alu.add)
                else:
                    nc.vector.tensor_scalar_add(kt_sb[:, h, sl], ps,
                                                bk_sb[:, h:h + 1])

            def v_group(xt_src, sm):
                sv = sm % (XC // P)
                psv = psum.tile([P, 512], dt.float32, tag="mm")
                for ht in range(HT):
                    nc.tensor.matmul(psv,
                                     xt_src[:, ht, sv * P:(sv + 1) * P],
                                     wv_sb[:, ht, :],
                                     start=(ht == 0), stop=(ht == HT - 1))
                nc.vector.tensor_copy(out=v_sb[:, sm, :], in_=psv)

            xt_prev = None
            for xc in range(NXC):
                if xc == 0:
                    xt_sb = xt0_sb
                    x8_sb = x80_sb if qk8 else None
                else:
                    if qk8:
                        x8_sb = x8pool.tile([P, HT, XC], dt.float8e4,
                                            tag="x8", name="x8_sb")
                        nc.sync.dma_start(x8_sb[:], xt8_r[:, xc, :, :])
                    xt_sb = xpool.tile([P, HT, XC], dt.bfloat16, tag="xt",
                                       name="xt_sb")
                    nc.sync.dma_start(xt_sb[:], xt_r[:, xc, :, :])
                sl = slice(xc * XC, (xc + 1) * XC)
                if xc == 0:
                    # the very first group would consume the startup DMA
                    # stream serially (1 matmul per arriving granule) and
                    # stall ~3.6us mid-accumulation; interleave all 4 heads
                    # by ht so each granule feeds 4 matmuls.  The extra 2
                    # PSUM banks borrow the st tag (free during phase 1).
                    psq4 = []
                    for h in range(G):
                        if h < 2:
                            psq4.append(psum.tile([P, 512], dt.float32,
                                                  tag="mm", name="psq_mm"))
                        else:
                            t = psum.tile([P, 2, QC], dt.float32, tag="st",
                                          name="psq_st")
                            psq4.append(t[:, 0, :])
                    for t in range(HT // 2):
                        for h in range(G):
                            qk_mms(psq4[h], wq_sb, h, x8_sb, xt_sb, Q_FP8,
                                   ht_step=t)
                    for h in range(G):
                        drain_q(h, sl, psq4[h])
                if xc == 0:
                    psk4 = []
                    for h in range(G):
                        if h < 2:
                            psk4.append(psum.tile([P, 512], dt.float32,
                                                  tag="mm", name="psk_mm"))
                        else:
                            t = psum.tile([P, 2, QC], dt.float32, tag="st",
                                          name="psk_st")
                            psk4.append(t[:, 0, :])
                    for t in range(HT // 2):
                        for h in range(G):
                            qk_mms(psk4[h], wk_sb, h, x8_sb, xt_sb, K_FP8,
                                   ht_step=t)
                    for h in range(G):
                        drain_k(h, sl, psk4[h])
                else:
                    # round-robin q/k/v groups: the fp8 DoubleRow work is
                    # interleaved with bf16 v-projections (~56% fp8 duty)
                    # instead of running as a dense block, to stay under
                    # the P0 power-downclock threshold.
                    # round-robin the previous xc's v-projections (their
                    # bf16 x arrives late in the HBM ramp; v isn't needed
                    # until phase 2, so each xc's v runs one chunk later)
                    # with this xc's fp8 q/k groups -- which also keeps the
                    # fp8 duty bursts ~4.4us, under the P0 power-downclock
                    # threshold that dense fp8 blocks trip.
                    for h in range(G):
                        if xc == 1 and h == 0:
                            # wv is still arriving here (~1.6us/granule vs
                            # 0.85us/granule consumption): split the first
                            # v-group's accumulation around the q-group so
                            # the granules have time to land.  The open
                            # PSUM borrows an idle st-tag bank.
                            tv = psum.tile([P, 2, QC], dt.float32,
                                           tag="st", name="psv_xc1")
                            psv0 = tv[:, 0, :]
                            for ht in range(HT // 2):
                                nc.tensor.matmul(psv0,
                                                 xt_prev[:, ht, 0:P],
                                                 wv_sb[:, ht, :],
                                                 start=(ht == 0), stop=False)
                            psq = psum.tile([P, 512], dt.float32, tag="mm")
                            qk_mms(psq, wq_sb, h, x8_sb, xt_sb, Q_FP8)
                            drain_q(h, sl, psq)
                            for ht in range(HT // 2, HT):
                                nc.tensor.matmul(psv0,
                                                 xt_prev[:, ht, 0:P],
                                                 wv_sb[:, ht, :],
                                                 start=False,
                                                 stop=(ht == HT - 1))
                            nc.vector.tensor_copy(out=v_sb[:, 0, :],
                                                  in_=psv0)
                        else:
                            v_group(xt_prev, (xc - 1) * (XC // P) + h)
                            psq = psum.tile([P, 512], dt.float32, tag="mm")
                            qk_mms(psq, wq_sb, h, x8_sb, xt_sb, Q_FP8)
                            drain_q(h, sl, psq)
                        psk = psum.tile([P, 512], dt.float32, tag="mm")
                        qk_mms(psk, wk_sb, h, x8_sb, xt_sb, K_FP8)
                        drain_k(h, sl, psk)
                xt_prev = xt_sb
            # phase-1 tail: the last xc's v-projections, with ST(0)+ST(1)
            # interleaved so exp warms up ~14us early and the qc0 iterations
            # (which have no out-proj filler) start with ACT ahead.
            for sv in range(XC // P):
                v_group(xt_prev, (NXC - 1) * (XC // P) + sv)
                for km in range(8 * sv, 8 * sv + 8, 2):
                    emit_st_pair(km // 16, km % 16)

            # out-proj weights: needed only from the first proj (~mid-kernel)
            wo_sb = wpool.tile([P, G, H], dt.bfloat16)
            nc.sync.dma_start(wo_sb[:], wot_r)

            # -------- Phase 2+3: attention + out-proj, uniform pipeline ----
            for i in range(NCH):
                h, qc = chunks[i]
                if i >= 1:
                    emit_znorm(i - 1)
                if 1 <= i < NCH - 1:
                    emit_st_pair(i + 1, 0)
                    emit_st_pair(i + 1, 2)
                emit_pv(i)
                # spread the previous qc's out-proj over this qc's 4
                # iterations (4 PSUM groups each); the DVE drain copies are
                # interleaved around the tree so the shared "mm" PSUM
                # rotation never blocks the tensor engine.
                pgs = list(range(4 * h, 4 * h + 4)) if qc >= 1 else []
                if i == NCH - 1:
                    for grp in pgs[:2]:
                        emit_proj_group(qc - 1, grp)
                    emit_tree_incremental(
                        i, [lambda g=g: emit_proj_group(qc - 1, g)
                            for g in pgs[2:]])
                else:
                    for grp in pgs[:2]:
                        emit_proj_group(qc - 1, grp)
                    emit_tree(i)
                    for grp in pgs[2:]:
                        emit_proj_group(qc - 1, grp)
                if 1 <= i < NCH - 1:
                    for km in range(4, KT, 2):
                        emit_st_pair(i + 1, km)
            emit_znorm(NCH - 1)
            for grp in range(16):
                emit_proj_group(NQC - 1, grp, last=True)

    nc.compile()
    return nc


def _get_nc():
    if "nc" not in _CACHE:
        _CACHE["nc"] = _build()
    return _CACHE["nc"]


def _make_in_maps(x, w_qkv, b_qkv, w_out):
    bf = ml_dtypes.bfloat16
    f8 = ml_dtypes.float8_e4m3     # IEEE e4m3, max 240 = TRN fp8_e4m3
    f32 = np.float32
    in_maps = []
    NXC = S // XC

    def pmaj(wT, dt_, scale=1.0):
        # [H, width] -> partition-major [128, HT, width]
        return np.ascontiguousarray(
            (wT * scale).reshape(HT, P, -1).transpose(1, 0, 2)).astype(dt_)

    for c in range(N_CORES):
        b = c // 4
        g = c % 4
        lo = GH * g
        hi = GH * (g + 1)
        xT = x[b].T                                    # [H, S]
        xsw = np.ascontiguousarray(
            xT.reshape(HT, P, NXC, XC).transpose(1, 2, 0, 3))
        xt = xsw.astype(bf)
        # fp8 q/k weights are pre-scaled x64 so |w|<=1.41 stays in e4m3
        # normals; the 1/64 is folded into the PSUM-drain tensor_scalar.
        wqt = pmaj(w_qkv[lo:hi, :].T, f8 if Q_FP8 else bf,
                   64.0 if Q_FP8 else 1.0)
        wkt = pmaj(w_qkv[H + lo:H + hi, :].T, f8 if K_FP8 else bf,
                   64.0 if K_FP8 else 1.0)
        wvt = pmaj(w_qkv[2 * H + lo:2 * H + hi, :].T, bf)
        bqs = np.ascontiguousarray(
            (b_qkv[lo:hi] * SCALE).astype(f32).reshape(G, P).T)
        bk = np.ascontiguousarray(
            b_qkv[H + lo:H + hi].astype(f32).reshape(G, P).T)
        wot = np.ascontiguousarray(
            w_out[:, lo:hi].T.reshape(G, P, H).transpose(1, 0, 2)).astype(bf)
        m = {"xt": xt, "wqt": wqt, "wkt": wkt, "wvt": wvt,
             "bqs": bqs, "bk": bk, "wot": wot}
        if Q_FP8 or K_FP8:
            m["xt8"] = xsw.astype(f8)
        in_maps.append(m)
    return in_maps


def kernel(x, w_qkv, b_qkv, w_out, b_out):
    import os
    import sys

    x = np.asarray(x, dtype=np.float32)
    w_qkv = np.asarray(w_qkv, dtype=np.float32)
    b_qkv = np.asarray(b_qkv, dtype=np.float32)
    w_out = np.asarray(w_out, dtype=np.float32)
    b_out = np.asarray(b_out, dtype=np.float32)

    from concourse.bass_utils import run_bass_kernel_spmd

    # NTFF tracing under axon needs the antenv.axon_hooks shim (test.py
    # installs it); without it a stray BASS_TRACE=1 in the environment would
    # crash the run — disable tracing in that case.
    if "antenv.axon_hooks" not in sys.modules:
        os.environ["BASS_NEVER_TRACE"] = "1"

    nc = _get_nc()
    in_maps = _make_in_maps(x, w_qkv, b_qkv, w_out)
    res = run_bass_kernel_spmd(nc, in_maps, core_ids=list(range(N_CORES)))
    _CACHE["last_results"] = res
    partials = [r["partial"] for r in res.results]

    bv = b_qkv[2 * H:3 * H]
    bias = b_out + w_out @ bv          # folded v-bias contribution
    out = np.empty((B, S, H), np.float32)
    for b in range(B):
        acc = partials[4 * b].astype(np.float32)
        for g in range(1, 4):
            acc += partials[4 * b + g].astype(np.float32)
        out[b] = acc + bias
    return out
